# revision 1
# baseline (speedup 1.0000x reference)
"""Trainium2 Bass kernel for causal self-attention with doubled rotary.

Full-input contract: kernel(**inputs) takes the complete tensors
(x [4,2048,2048], wq/wk/wv/wo [2048,2048]) and returns [4,2048,2048] fp32.

Sharding: 8 cores = 4 batch elements x 2 head-halves (8 heads each).
Each core computes a partial output projection (its heads' columns of wo);
the host sums the two partials per batch element.

Per-core structure (engine streams execute in emission order, so independent
work is interleaved at emission time to keep the PE dense):
  - group g in 0..3 owns heads (2g, 2g+1): projections q/k/v (contraction
    over embd, fp32r matmuls), doubled-angle rotary on DVE (the reference
    applies rotary twice; R(t)^2 == R(2t)), all into double-buffered SBUF
    tiles.
  - attention pair g-1 is emitted interleaved with the projection of group
    g: QK^T computed transposed (ST[s,t]) so exp(ST) feeds the PV matmul
    directly with v as the stationary operand — no transposes. Causal
    diagonal chunks are sliced at the 128-column grid, with a single
    [128,128] triangular mask. Softmax denominators accumulate on DVE with
    the final add fused into the fp32r cast; an all-ones [128,128] matmul
    does the partition reduce + broadcast in one shot; full-width
    reciprocal; normalization rides the PSUM->SBUF copy of the PV output.
    yT goes to a DRAM spill for the output projection.
  - the last pair is interleaved with the first half of the output
    projection; the rest of the output projection follows.

All matmul operands are float32r (fp32 rounded to 11 mantissa bits, 4x the
fp32 PE rate). The host pre-rounds DRAM-fed operands; on-chip producers
(ACT exp, DVE copies/adds) round at their outputs.
"""

import os
import sys

for _p in ("/opt/trn_rl_repo", "/root/.axon_site/_ro/trn_rl_repo"):
    if os.path.isdir(_p) and _p not in sys.path:
        sys.path.insert(0, _p)

import numpy as np

import concourse.bass as bass
import concourse.mybir as mybir
from concourse import bacc
from concourse.bass import ds
from concourse.tile import TileContext
from concourse.bass_utils import run_bass_kernel_spmd

F32 = mybir.dt.float32
F32R = mybir.dt.float32r
BF16 = mybir.dt.bfloat16
FP16 = mybir.dt.float16

P = 128          # partitions / head dim
T = 2048         # sequence length
E = 2048         # embedding dim
B = 4
HPC = 8          # heads per core
D = 128          # head dim
PAN = 512        # attention t-panel width (PSUM bank limit for fp32)
NPAN = T // PAN  # 4
XPAN = 256       # projection t-panel width
NXP = T // XPAN  # 8
EO = E // P      # 16 contraction chunks for projections
NGRP = 4         # head pairs per core
NCH = T // P     # 16 s-chunks (also v t-tiles)
SCALE = 1.0 / float(np.sqrt(D))
NEG = -1.0e9

ADD = mybir.AluOpType.add
MULT = mybir.AluOpType.mult
EXP = mybir.ActivationFunctionType.Exp


def _zip_emit(*lists):
    """Emit thunks from several lists round-robin, proportionally."""
    lists = [list(l) for l in lists if l]
    if not lists:
        return
    total = max(len(l) for l in lists)
    idx = [0.0] * len(lists)
    step = [len(l) / total for l in lists]
    for _ in range(total):
        for li, l in enumerate(lists):
            idx[li] += step[li]
            while idx[li] >= 1.0 and l:
                l.pop(0)()
                idx[li] -= 1.0
    for l in lists:
        for f in l:
            f()


class Ctx:
    pass


def build_program():
    nc = bacc.Bacc()
    cx = Ctx()
    cx.nc = nc

    cx.xT = nc.declare_dram_parameter("xT", [E, T], F32R, isOutput=False)
    cx.wqT = nc.declare_dram_parameter("wqT", [E, HPC * D], F32R, isOutput=False)
    cx.wkT = nc.declare_dram_parameter("wkT", [E, HPC * D], F32R, isOutput=False)
    cx.wvT = nc.declare_dram_parameter("wvT", [E, HPC * D], F32R, isOutput=False)
    cx.woT = nc.declare_dram_parameter("woT", [HPC * D, E], F32R, isOutput=False)
    cx.cos2 = nc.declare_dram_parameter("cos2", [P, T], FP16, isOutput=False)
    cx.sin2 = nc.declare_dram_parameter("sin2", [P, T], FP16, isOutput=False)
    cx.mask = nc.declare_dram_parameter("mask", [P, P], BF16, isOutput=False)
    cx.out = nc.declare_dram_parameter("out", [E, T], F32, isOutput=True)
    cx.ytd = nc.dram_tensor("yt_scratch", [HPC * D, T], F32R)

    with TileContext(nc) as tc:
        cx.tc = tc
        with tc.tile_pool(name="const", bufs=1) as cpool:
            cx.mk = cpool.tile([P, P], BF16, tag="mk")
            nc.sync.dma_start(cx.mk, cx.mask[:, :])
            om_f = cpool.tile([P, P], F32, tag="om_f")
            nc.vector.memset(om_f, 1.0)
            cx.onesmat = cpool.tile([P, P], F32R, tag="onesmat")
            nc.scalar.copy(cx.onesmat, om_f)

            with (
                tc.tile_pool(name="ex", bufs=4) as expool,
                tc.tile_pool(name="dn1", bufs=2) as dn1pool,
                tc.tile_pool(name="yts", bufs=2) as ytpool,
                tc.tile_pool(name="psS", bufs=2, space="PSUM") as psS,
                tc.tile_pool(name="psY", bufs=2, space="PSUM") as psY,
                tc.tile_pool(name="psD", bufs=2, space="PSUM") as psD,
                tc.tile_pool(name="qk", bufs=2) as qkpool,
                tc.tile_pool(name="vp", bufs=2) as vpool,
            ):
                cx.expool, cx.dn1pool, cx.ytpool = expool, dn1pool, ytpool
                cx.psS, cx.psY, cx.psD = psS, psY, psD
                cx.qkpool, cx.vpool = qkpool, vpool
                cx.qkv = {}  # g -> (qT, kT, v_sb)

                with (
                    tc.tile_pool(name="tab", bufs=1) as tabpool,
                    tc.tile_pool(name="xp", bufs=2) as xpool,
                    tc.tile_pool(name="wp", bufs=1) as wpool,
                    tc.tile_pool(name="rot", bufs=2) as rotpool,
                    tc.tile_pool(name="sw", bufs=1) as swpool,
                    tc.tile_pool(name="psP", bufs=2, space="PSUM") as psP,
                ):
                    cx.xpool, cx.wpool = xpool, wpool
                    cx.rotpool, cx.swpool, cx.psP = rotpool, swpool, psP
                    cx.c2 = tabpool.tile([P, T], FP16, tag="c2")
                    nc.sync.dma_start(cx.c2, cx.cos2[:, :])
                    cx.s2 = tabpool.tile([P, T], FP16, tag="s2")
                    nc.sync.dma_start(cx.s2, cx.sin2[:, :])

                    for f in _proj_thunks(cx, 0):
                        f()
                    for g in range(1, NGRP):
                        _zip_emit(_proj_thunks(cx, g), _attn_thunks(cx, g - 1))

                with (
                    tc.tile_pool(name="wo", bufs=1) as wopool,
                    tc.tile_pool(name="yl", bufs=2) as ylpool,
                    tc.tile_pool(name="ob", bufs=3) as opool,
                    tc.tile_pool(name="psO", bufs=2, space="PSUM") as psO,
                ):
                    cx.wopool, cx.ylpool, cx.opool, cx.psO = (
                        wopool, ylpool, opool, psO)
                    # wo half 0 resident during the last attention pair
                    cx.wo_half = {}
                    _load_wo_half(cx, 0)
                    # outproj(jp) may only be emitted after pair-3 has
                    # finalized panel jp (it reads ytd rows for heads 6/7):
                    # interleave panel jp's outproj with panel jp+1's chunks.
                    panels = [_attn_thunks(cx, NGRP - 1, only_jp=jp)
                              for jp in range(NPAN)]
                    oproj0 = [_outproj_thunks(cx, 0, only_jp=jp)
                              for jp in range(NPAN)]
                    for f in panels[0]:
                        f()
                    for jp in range(1, NPAN):
                        _zip_emit(panels[jp], oproj0[jp - 1])
                    for f in oproj0[NPAN - 1]:
                        f()
                    _load_wo_half(cx, 1)
                    for f in _outproj_thunks(cx, 1):
                        f()

    nc.finalize()
    return nc


def _proj_thunks(cx, g):
    """Thunk list for group g's projections + rotary (no spill)."""
    nc = cx.nc
    thunks = []

    def start_group():
        wq_sb = cx.wpool.tile([P, EO, 2 * D], F32R, tag="wq")
        nc.sync.dma_start(
            wq_sb,
            cx.wqT.rearrange("(eo p) d -> p eo d", p=P)[:, :, ds(g * 2 * D, 2 * D)],
        )
        wk_sb = cx.wpool.tile([P, EO, 2 * D], F32R, tag="wk")
        nc.sync.dma_start(
            wk_sb,
            cx.wkT.rearrange("(eo p) d -> p eo d", p=P)[:, :, ds(g * 2 * D, 2 * D)],
        )
        wv_sb = cx.wpool.tile([P, EO, 2 * D], F32R, tag="wv")
        nc.sync.dma_start(
            wv_sb,
            cx.wvT.rearrange("(eo p) d -> p eo d", p=P)[:, :, ds(g * 2 * D, 2 * D)],
        )
        qT = cx.qkpool.tile([P, 2, T], F32R, tag="qT")
        kT = cx.qkpool.tile([P, 2, T], F32R, tag="kT")
        v_sb = cx.vpool.tile([P, NCH, 2 * D], F32R, tag="v")
        cx.qkv[g] = (qT, kT, v_sb)
        cx._w = (wq_sb, wk_sb, wv_sb)

    thunks.append(start_group)

    state = {}

    def load_panel(xj):
        def f():
            xp = cx.xpool.tile([P, EO, XPAN], F32R, tag="xp")
            nc.sync.dma_start(
                xp,
                cx.xT.rearrange("(eo p) t -> p eo t", p=P)[:, :, ds(xj * XPAN, XPAN)],
            )
            state[xj] = xp
        return f

    def qk_group(xj, wi, hl):
        def f():
            xp = state[xj]
            w_sb = cx._w[wi]
            dst = cx.qkv[g][wi]
            ps = cx.psP.tile([P, PAN], F32, tag="psP")
            psq = ps[:, :XPAN]
            for eo in range(EO):
                nc.tensor.matmul(
                    psq,
                    lhsT=w_sb[:, eo, ds(hl * D, D)],
                    rhs=xp[:, eo, :],
                    start=(eo == 0),
                    stop=(eo == EO - 1),
                )
            nc.vector.tensor_copy(dst[:, hl, ds(xj * XPAN, XPAN)], psq)
        return f

    def v_group(xj, tt):
        def f():
            xp = state[xj]
            wv_sb = cx._w[2]
            v_sb = cx.qkv[g][2]
            ps = cx.psP.tile([P, PAN], F32, tag="psP")
            psv = ps[:, : 2 * D]
            for eo in range(EO):
                nc.tensor.matmul(
                    psv,
                    lhsT=xp[:, eo, ds(tt * P, P)],
                    rhs=wv_sb[:, eo, :],
                    start=(eo == 0),
                    stop=(eo == EO - 1),
                )
            nc.vector.tensor_copy(v_sb[:, xj * (XPAN // P) + tt, :], psv)
        return f

    def rot_panel(src_i, hl, jp):
        def f():
            src = cx.qkv[g][src_i]
            sl = ds(jp * PAN, PAN)
            qsw = cx.swpool.tile([P, PAN], F32R, tag="qsw")
            nc.sync.dma_start(qsw[0:64, :], src[64:128, hl, sl])
            nc.sync.dma_start(qsw[64:128, :], src[0:64, hl, sl])
            tmp = cx.rotpool.tile([P, PAN], F32, tag="rtmp")
            nc.vector.tensor_tensor(tmp, qsw[:, :], cx.s2[:, sl], op=MULT)
            nc.vector.tensor_tensor(
                src[:, hl, sl], src[:, hl, sl], cx.c2[:, sl], op=MULT
            )
            nc.vector.tensor_tensor(src[:, hl, sl], src[:, hl, sl], tmp, op=ADD)
        return f

    for xj in range(NXP):
        thunks.append(load_panel(xj))
        for wi in range(2):
            for hl in range(2):
                thunks.append(qk_group(xj, wi, hl))
        for tt in range(XPAN // P):
            thunks.append(v_group(xj, tt))
        if xj % 2 == 1:
            jp = xj // 2
            for src_i in range(2):
                for hl in range(2):
                    thunks.append(rot_panel(src_i, hl, jp))
    return thunks


def _attn_thunks(cx, g, only_jp=None):
    """Thunk list for the attention of head pair g (heads 2g, 2g+1)."""
    nc = cx.nc
    thunks = []
    st8 = cx.__dict__.setdefault(f"_attn_state_{g}", {})

    def chunk(hl, jp, i):
        def f():
            qT, kT, v_sb = cx.qkv[g]
            nch = 4 * jp + 4
            if i == 0:
                ytp = cx.psY.tile([P, PAN], F32, tag="psY")
                dps = cx.psD.tile([P, PAN], F32, tag="psD")
                st8[(hl, jp)] = (ytp, dps)
            ytp, dps = st8[(hl, jp)]
            di = i - 4 * jp
            off = P * di if di > 0 else 0
            w = PAN - off
            st = cx.psS.tile([P, PAN], F32, tag="psS")
            stw = st[:, off:PAN]
            nc.tensor.matmul(
                stw,
                lhsT=kT[:, hl, ds(i * P, P)],
                rhs=qT[:, hl, ds(jp * PAN + off, w)],
                start=True,
                stop=True,
            )
            if di >= 0:
                nc.vector.tensor_tensor(
                    st[:, off:off + P], st[:, off:off + P], cx.mk, op=ADD
                )
            ex = cx.expool.tile([P, PAN], F32R, tag="ex")
            exw = ex[:, off:PAN]
            nc.scalar.activation(exw, stw, EXP, scale=SCALE)
            last = i == nch - 1
            nc.tensor.matmul(
                ytp[:, off:PAN],
                lhsT=v_sb[:, i, ds(hl * D, D)],
                rhs=exw,
                start=(i == 0),
                stop=last,
            )
            # denominator: reduce+broadcast over s via the all-ones matrix,
            # accumulated in PSUM across chunks
            nc.tensor.matmul(
                dps[:, off:PAN],
                lhsT=cx.onesmat,
                rhs=exw,
                start=(i == 0),
                stop=last,
            )
        return f

    def finalize(hl, jp):
        def f():
            h = 2 * g + hl
            ytp, dps = st8.pop((hl, jp))
            rdb = cx.dn1pool.tile([P, PAN], F32, tag="rdb")
            nc.vector.reciprocal_approx_fast(out=rdb, in_=dps)
            yts = cx.ytpool.tile([P, PAN], F32R, tag="yts")
            nc.vector.tensor_tensor(yts, ytp, rdb, op=MULT)
            nc.gpsimd.dma_start(
                cx.ytd[ds(h * D, D), ds(jp * PAN, PAN)], yts
            )
        return f

    jps = range(NPAN) if only_jp is None else [only_jp]
    for jp in jps:
        nch = 4 * jp + 4
        for i in range(nch):
            for hl in range(2):
                thunks.append(chunk(hl, jp, i))
        for hl in range(2):
            thunks.append(finalize(hl, jp))
    return thunks


def _load_wo_half(cx, half):
    nc = cx.nc
    wo_sb = cx.wopool.tile([P, HPC, E // 2], F32R, tag="wo")
    nc.sync.dma_start(
        wo_sb,
        cx.woT.rearrange("(c p) e -> p c e", p=P)[:, :, ds(half * (E // 2), E // 2)],
    )
    cx.wo_half[half] = wo_sb


def _outproj_thunks(cx, half, only_jp=None):
    """Thunk list for the output projection over e-tiles of one wo half."""
    nc = cx.nc
    thunks = []
    yls = cx.__dict__.setdefault(f"_yl_state_{half}", {})

    def load_yl(jp):
        def f():
            yl = cx.ylpool.tile([P, HPC, PAN], F32R, tag="yl")
            nc.sync.dma_start(
                yl,
                cx.ytd.rearrange("(c p) t -> p c t", p=P)[:, :, ds(jp * PAN, PAN)],
            )
            yls[jp] = yl
        return f

    def etile(jp, et):
        def f():
            wo_sb = cx.wo_half[half]
            yl = yls[jp]
            ps = cx.psO.tile([P, PAN], F32, tag="psO")
            for dc in range(HPC):
                nc.tensor.matmul(
                    ps,
                    lhsT=wo_sb[:, dc, ds((et - half * 8) * P, P)],
                    rhs=yl[:, dc, :],
                    start=(dc == 0),
                    stop=(dc == HPC - 1),
                )
            ob = cx.opool.tile([P, PAN], F32, tag="ob")
            nc.vector.tensor_copy(ob, ps)
            nc.gpsimd.dma_start(
                cx.out[ds(et * P, P), ds(jp * PAN, PAN)], ob
            )
        return f

    jps = range(NPAN) if only_jp is None else [only_jp]
    for jp in jps:
        thunks.append(load_yl(jp))
        for et in range(half * 8, half * 8 + 8):
            thunks.append(etile(jp, et))
    return thunks


def round_f32r(a):
    """Round fp32 to the fp32r grid (11 mantissa bits, RNE) — matches the
    compiler's cast_fp32_to_fp32r bit-for-bit."""
    b = np.ascontiguousarray(a, dtype=np.float32).view(np.uint32).astype(np.uint64)
    lsb = (b >> 12) & 1
    b2 = (b + 0x7FF + lsb) & ~np.uint64(0xFFF)
    return b2.astype(np.uint32).view(np.float32).reshape(a.shape)


def make_tables():
    j = np.arange(0, D, 2, dtype=np.float64) / D
    inv_freq = 1.0 / (10000.0 ** j)
    t = np.arange(T, dtype=np.float64)
    fr = np.outer(t, inv_freq)                            # [T, 64]
    c2 = np.cos(2.0 * fr).T                               # [64, T]
    s2 = np.sin(2.0 * fr).T
    cos2 = np.concatenate([c2, c2], axis=0).astype(np.float16)
    sin2 = np.concatenate([s2, -s2], axis=0).astype(np.float16)
    return cos2, sin2


def make_mask():
    import ml_dtypes
    s = np.arange(P)[:, None]
    c = np.arange(P)[None, :]
    return np.where(s <= c, 0.0, NEG).astype(ml_dtypes.bfloat16)


def make_in_maps(x, wq, wk, wv, wo):
    cos2, sin2 = make_tables()
    mask = make_mask()
    in_maps = []
    for c in range(8):
        b, hh = c // 2, c % 2
        rows = slice(hh * HPC * D, (hh + 1) * HPC * D)
        in_maps.append({
            "xT": round_f32r(x[b].T),
            "wqT": round_f32r(wq[rows].T),
            "wkT": round_f32r(wk[rows].T),
            "wvT": round_f32r(wv[rows].T),
            "woT": round_f32r(wo[:, rows].T),
            "cos2": cos2,
            "sin2": sin2,
            "mask": mask,
        })
    return in_maps


_PROGRAM_CACHE = {}


def get_program():
    if "nc" not in _PROGRAM_CACHE:
        _PROGRAM_CACHE["nc"] = build_program()
    return _PROGRAM_CACHE["nc"]


def kernel(x, wq, wk, wv, wo, _results_hook=None):
    x = np.asarray(x, dtype=np.float32)
    wq = np.asarray(wq, dtype=np.float32)
    wk = np.asarray(wk, dtype=np.float32)
    wv = np.asarray(wv, dtype=np.float32)
    wo = np.asarray(wo, dtype=np.float32)

    nc = get_program()
    in_maps = make_in_maps(x, wq, wk, wv, wo)
    res = run_bass_kernel_spmd(nc, in_maps, list(range(8)))
    if _results_hook is not None:
        _results_hook(res)
    outs = [r["out"] for r in res.results]
    full = np.empty((B, T, E), dtype=np.float32)
    for b in range(B):
        full[b] = (outs[2 * b] + outs[2 * b + 1]).T
    return full



# revision 3
# speedup vs baseline: 1.2393x; 1.2393x over previous
"""Trainium2 Bass kernel for causal self-attention with doubled rotary.

Full-input contract: kernel(**inputs) takes the complete tensors
(x [4,2048,2048], wq/wk/wv/wo [2048,2048]) and returns [4,2048,2048] fp32.

Sharding: 8 cores = 4 batch elements x 2 head-halves (8 heads each).
Each core computes a partial output projection (its heads' columns of wo);
the host sums the two partials per batch element.

All matmul operands are bf16 (fp8 exceeds the error budget on every path —
measured 2.8-4.7e-2 vs the 2e-2 gate; all-bf16 lands at ~3.6e-3). bf16
halves DMA bytes vs fp32r and enables FWL weight loads.

Per-core structure (engine streams execute in emission order; independent
work is interleaved at emission time to keep the PE dense):
  - phase 0: two sweeps over x panels (512-wide). Sweep A: q/k projections
    of group 0 + V projection (all 8 heads, low column half). Sweep B:
    V high half + doubled-angle rotary (R(t)^2 == R(2t)) for group 0.
  - phases 1..3: q/k projections + rotary of group g interleaved with
    attention of head pair g-1. Attention is computed transposed (ST[s,t])
    so exp(ST) feeds the PV matmul directly with v stationary.
  - softmax denominator: DVE accumulates the bf16 exp chunks into an f32r
    panel accumulator; ONE all-ones matmul per (head, panel) does the
    partition reduce + broadcast (vs one matmul per chunk).
  - y stays resident in SBUF (no DRAM spill); the output projection reads
    it directly, interleaved with the last attention pair.
"""

import os
import sys

for _p in ("/opt/trn_rl_repo", "/root/.axon_site/_ro/trn_rl_repo"):
    if os.path.isdir(_p) and _p not in sys.path:
        sys.path.insert(0, _p)

import numpy as np

import concourse.bass as bass
import concourse.mybir as mybir
from concourse import bacc
from concourse.bass import ds
from concourse.tile import TileContext
from concourse.bass_utils import run_bass_kernel_spmd

F32 = mybir.dt.float32
F32R = mybir.dt.float32r
BF16 = mybir.dt.bfloat16
FP16 = mybir.dt.float16

P = 128          # partitions / head dim
T = 2048         # sequence length
E = 2048         # embedding dim
B = 4
HPC = 8          # heads per core
D = 128          # head dim
PAN = 512        # panel width (PSUM bank limit for fp32)
NPAN = T // PAN  # 4
EO = E // P      # 16 contraction chunks for projections
EQ = 4           # eo chunks per input-DMA quarter
NGRP = 4         # head pairs per core
NCH = T // P     # 16 s-chunks (also v t-tiles)
SCALE = 1.0 / float(np.sqrt(D))
NEG = -1.0e9

ADD = mybir.AluOpType.add
MULT = mybir.AluOpType.mult
EXP = mybir.ActivationFunctionType.Exp


def _zip_emit(*lists):
    """Emit thunks from several lists round-robin, proportionally."""
    lists = [list(l) for l in lists if l]
    if not lists:
        return
    total = max(len(l) for l in lists)
    idx = [0.0] * len(lists)
    step = [len(l) / total for l in lists]
    for _ in range(total):
        for li, l in enumerate(lists):
            idx[li] += step[li]
            while idx[li] >= 1.0 and l:
                l.pop(0)()
                idx[li] -= 1.0
    for l in lists:
        for f in l:
            f()


class Ctx:
    pass


def _dma_quarters(nc, dst, src_re):
    """Split a [P, EO, W] load into EO/EQ quarter DMAs for early starts."""
    for qq in range(EO // EQ):
        nc.sync.dma_start(
            dst[:, ds(qq * EQ, EQ), :], src_re[:, ds(qq * EQ, EQ), :]
        )


def build_program():
    nc = bacc.Bacc()
    cx = Ctx()
    cx.nc = nc

    cx.xT = nc.declare_dram_parameter("xT", [E, T], BF16, isOutput=False)
    cx.wqT = nc.declare_dram_parameter("wqT", [E, HPC * D], BF16, isOutput=False)
    cx.wkT = nc.declare_dram_parameter("wkT", [E, HPC * D], BF16, isOutput=False)
    cx.wvT = nc.declare_dram_parameter("wvT", [E, HPC * D], BF16, isOutput=False)
    cx.woT = nc.declare_dram_parameter("woT", [HPC * D, E], BF16, isOutput=False)
    cx.cos2 = nc.declare_dram_parameter("cos2", [P, T], FP16, isOutput=False)
    cx.sin2 = nc.declare_dram_parameter("sin2", [P, T], FP16, isOutput=False)
    cx.mask = nc.declare_dram_parameter("mask", [P, P], BF16, isOutput=False)
    cx.out = nc.declare_dram_parameter("out", [E, T], F32, isOutput=True)

    with TileContext(nc) as tc:
        cx.tc = tc
        with tc.tile_pool(name="const", bufs=1) as cpool:
            om_f = cpool.tile([P, P], F32, tag="om_f")
            nc.vector.memset(om_f, 1.0)
            cx.onesmat = cpool.tile([P, P], F32R, tag="onesmat")
            nc.scalar.copy(cx.onesmat, om_f)
            cx.mk = cpool.tile([P, P], BF16, tag="mk")

            with (
                tc.tile_pool(name="ex", bufs=3) as expool,
                tc.tile_pool(name="acc", bufs=3) as accpool,
                tc.tile_pool(name="dn", bufs=2) as dnpool,
                tc.tile_pool(name="qk", bufs=2) as qkpool,
                tc.tile_pool(name="vp", bufs=1) as vpool,
                tc.tile_pool(name="yp", bufs=1) as ypool,
                tc.tile_pool(name="psS", bufs=3, space="PSUM") as psS,
                tc.tile_pool(name="psY", bufs=2, space="PSUM") as psY,
            ):
                cx.expool, cx.accpool, cx.dnpool = expool, accpool, dnpool
                cx.qkpool = qkpool
                cx.v_sb = vpool.tile([P, NCH, HPC * D], BF16, tag="v")
                cx.y_sb = ypool.tile([P, HPC, T], BF16, tag="y")
                cx.psS, cx.psY = psS, psY
                cx.qkv = {}  # g -> (qT, kT)

                with (
                    tc.tile_pool(name="tab", bufs=1) as tabpool,
                    tc.tile_pool(name="xp", bufs=2) as xpool,
                    tc.tile_pool(name="wqk", bufs=2) as wqkpool,
                    tc.tile_pool(name="wv", bufs=1) as wvpool,
                    tc.tile_pool(name="rot", bufs=1) as rotpool,
                    tc.tile_pool(name="sw", bufs=2) as swpool,
                    tc.tile_pool(name="psP", bufs=3, space="PSUM") as psP,
                ):
                    cx.xpool, cx.wqkpool, cx.wvpool = xpool, wqkpool, wvpool
                    cx.rotpool, cx.swpool, cx.psP = rotpool, swpool, psP

                    # phase 0 sweep A: q/k of group 0 + v low half
                    for f in _proj_thunks(cx, 0, v_half=0):
                        f()
                    # tables + mask land during sweep B
                    cx.c2 = tabpool.tile([P, T], FP16, tag="c2")
                    nc.sync.dma_start(cx.c2, cx.cos2[:, :])
                    cx.s2 = tabpool.tile([P, T], FP16, tag="s2")
                    nc.sync.dma_start(cx.s2, cx.sin2[:, :])
                    nc.sync.dma_start(cx.mk, cx.mask[:, :])
                    # phase 0 sweep B: v high half + rotary of group 0
                    _zip_emit(_vsweep_thunks(cx, v_half=1), _rot_thunks(cx, 0))

                    for g in range(1, NGRP):
                        pj = _proj_thunks(cx, g)
                        rj = _rot_thunks(cx, g)
                        _zip_emit(pj + rj, _attn_thunks(cx, g - 1))

                with (
                    tc.tile_pool(name="wo", bufs=1) as wopool,
                    tc.tile_pool(name="ob", bufs=3) as opool,
                    tc.tile_pool(name="psO", bufs=2, space="PSUM") as psO,
                ):
                    cx.opool, cx.psO = opool, psO
                    cx.wo_sb = wopool.tile([P, HPC, E], BF16, tag="wo")
                    # low e-half first so outproj can start early
                    for half in range(2):
                        nc.sync.dma_start(
                            cx.wo_sb[:, :, ds(half * (E // 2), E // 2)],
                            cx.woT.rearrange("(c p) e -> p c e", p=P)[
                                :, :, ds(half * (E // 2), E // 2)
                            ],
                        )
                    panels = [_attn_thunks(cx, NGRP - 1, only_jp=jp)
                              for jp in range(NPAN)]
                    oproj = [_outproj_thunks(cx, jp) for jp in range(NPAN)]
                    for f in panels[0]:
                        f()
                    for jp in range(1, NPAN):
                        _zip_emit(panels[jp], oproj[jp - 1])
                    for f in oproj[NPAN - 1]:
                        f()

    nc.finalize()
    return nc


def _load_panel(cx, xj, state):
    def f():
        xp = cx.xpool.tile([P, EO, PAN], BF16, tag="xp")
        _dma_quarters(
            cx.nc, xp,
            cx.xT.rearrange("(eo p) t -> p eo t", p=P)[:, :, ds(xj * PAN, PAN)],
        )
        state[xj] = xp
    return f


def _load_wv_half(cx, half):
    def f():
        wv_sb = cx.wvpool.tile([P, EO, HPC * D // 2], BF16, tag="wv")
        _dma_quarters(
            cx.nc, wv_sb,
            cx.wvT.rearrange("(eo p) d -> p eo d", p=P)[
                :, :, ds(half * HPC * D // 2, HPC * D // 2)
            ],
        )
        cx._wv = wv_sb
    return f


def _v_group(cx, state, xj, tt, half):
    """v for all 8 heads, one s-chunk, one 512-column half."""
    def f():
        nc = cx.nc
        xp = state[xj]
        ps = cx.psP.tile([P, PAN], F32, tag="psP")
        for eo in range(EO):
            nc.tensor.matmul(
                ps,
                lhsT=xp[:, eo, ds(tt * P, P)],
                rhs=cx._wv[:, eo, :],
                start=(eo == 0),
                stop=(eo == EO - 1),
            )
        nc.scalar.copy(
            cx.v_sb[:, xj * (PAN // P) + tt, ds(half * PAN, PAN)], ps
        )
    return f


def _proj_thunks(cx, g, v_half=None):
    """Thunks for group g's q/k projections (+ v half during phase 0)."""
    nc = cx.nc
    thunks = []
    state = {}

    thunks.append(_load_panel(cx, 0, state))

    def start_group():
        wq_sb = cx.wqkpool.tile([P, EO, 2 * D], BF16, tag="wq")
        _dma_quarters(
            nc, wq_sb,
            cx.wqT.rearrange("(eo p) d -> p eo d", p=P)[:, :, ds(g * 2 * D, 2 * D)],
        )
        wk_sb = cx.wqkpool.tile([P, EO, 2 * D], BF16, tag="wk")
        _dma_quarters(
            nc, wk_sb,
            cx.wkT.rearrange("(eo p) d -> p eo d", p=P)[:, :, ds(g * 2 * D, 2 * D)],
        )
        qT = cx.qkpool.tile([P, 2, T], BF16, tag="qT")
        kT = cx.qkpool.tile([P, 2, T], BF16, tag="kT")
        cx.qkv[g] = (qT, kT)
        cx._w = (wq_sb, wk_sb)

    thunks.append(start_group)
    if v_half is not None:
        thunks.append(_load_wv_half(cx, v_half))

    def qk_group(xj, wi, hl):
        def f():
            xp = state[xj]
            w_sb = cx._w[wi]
            dst = cx.qkv[g][wi]
            ps = cx.psP.tile([P, PAN], F32, tag="psP")
            for eo in range(EO):
                nc.tensor.matmul(
                    ps,
                    lhsT=w_sb[:, eo, ds(hl * D, D)],
                    rhs=xp[:, eo, :],
                    start=(eo == 0),
                    stop=(eo == EO - 1),
                )
            nc.vector.tensor_copy(dst[:, hl, ds(xj * PAN, PAN)], ps)
        return f

    for xj in range(NPAN):
        if xj + 1 < NPAN:
            thunks.append(_load_panel(cx, xj + 1, state))
        for wi in range(2):
            for hl in range(2):
                thunks.append(qk_group(xj, wi, hl))
        if v_half is not None:
            for tt in range(PAN // P):
                thunks.append(_v_group(cx, state, xj, tt, v_half))
    return thunks


def _vsweep_thunks(cx, v_half):
    """Second phase-0 sweep: reload x panels, compute the other v half."""
    thunks = []
    state = {}
    thunks.append(_load_wv_half(cx, v_half))
    thunks.append(_load_panel(cx, 0, state))
    for xj in range(NPAN):
        if xj + 1 < NPAN:
            thunks.append(_load_panel(cx, xj + 1, state))
        for tt in range(PAN // P):
            thunks.append(_v_group(cx, state, xj, tt, v_half))
    return thunks


def _rot_thunks(cx, g):
    """Doubled-angle rotary on group g's qT/kT, one 512-panel at a time."""
    nc = cx.nc
    thunks = []

    def rot_panel(src_i, hl, jp):
        def f():
            src = cx.qkv[g][src_i]
            sl = ds(jp * PAN, PAN)
            qsw = cx.swpool.tile([P, PAN], BF16, tag="qsw")
            nc.gpsimd.dma_start(qsw[0:64, :], src[64:128, hl, sl])
            nc.gpsimd.dma_start(qsw[64:128, :], src[0:64, hl, sl])
            tmp = cx.rotpool.tile([P, PAN], F32, tag="rtmp")
            nc.vector.tensor_tensor(tmp, qsw[:, :], cx.s2[:, sl], op=MULT)
            nc.vector.tensor_tensor(
                src[:, hl, sl], src[:, hl, sl], cx.c2[:, sl], op=MULT
            )
            nc.vector.tensor_tensor(src[:, hl, sl], src[:, hl, sl], tmp, op=ADD)
        return f

    for jp in range(NPAN):
        for src_i in range(2):
            for hl in range(2):
                thunks.append(rot_panel(src_i, hl, jp))
    return thunks


def _attn_thunks(cx, g, only_jp=None):
    """Thunk list for the attention of head pair g (heads 2g, 2g+1)."""
    nc = cx.nc
    thunks = []
    st8 = cx.__dict__.setdefault(f"_attn_state_{g}", {})

    def chunk(hl, jp, i):
        def f():
            qT, kT = cx.qkv[g]
            nch = 4 * jp + 4
            if i == 0:
                ytp = cx.psY.tile([P, PAN], F32, tag="psY")
                acc = cx.accpool.tile([P, PAN], F32R, tag="acc")
                st8[(hl, jp)] = (ytp, acc)
            ytp, acc = st8[(hl, jp)]
            di = i - 4 * jp
            off = P * di if di > 0 else 0
            w = PAN - off
            st = cx.psS.tile([P, PAN], F32, tag="psS")
            stw = st[:, off:PAN]
            nc.tensor.matmul(
                stw,
                lhsT=kT[:, hl, ds(i * P, P)],
                rhs=qT[:, hl, ds(jp * PAN + off, w)],
                start=True,
                stop=True,
            )
            if di >= 0:
                nc.vector.tensor_tensor(
                    st[:, off:off + P], st[:, off:off + P], cx.mk, op=ADD
                )
            ex = cx.expool.tile([P, PAN], BF16, tag="ex")
            exw = ex[:, off:PAN]
            nc.scalar.activation(exw, stw, EXP, scale=SCALE)
            last = i == nch - 1
            nc.tensor.matmul(
                ytp[:, off:PAN],
                lhsT=cx.v_sb[:, i, ds((2 * g + hl) * D, D)],
                rhs=exw,
                start=(i == 0),
                stop=last,
            )
            if i == 0:
                nc.vector.tensor_copy(acc, ex)
            else:
                nc.vector.tensor_tensor(
                    acc[:, off:PAN], acc[:, off:PAN], exw, op=ADD
                )
        return f

    def finalize(hl, jp):
        def f():
            h = 2 * g + hl
            ytp, acc = st8.pop((hl, jp))
            dps = cx.psS.tile([P, PAN], F32, tag="psS")
            nc.tensor.matmul(
                dps, lhsT=cx.onesmat, rhs=acc, start=True, stop=True
            )
            rdb = cx.dnpool.tile([P, PAN], F32, tag="rdb")
            nc.vector.reciprocal_approx_fast(out=rdb, in_=dps)
            nc.vector.tensor_tensor(
                cx.y_sb[:, h, ds(jp * PAN, PAN)], ytp, rdb, op=MULT
            )
        return f

    jps = range(NPAN) if only_jp is None else [only_jp]
    for jp in jps:
        nch = 4 * jp + 4
        for i in range(nch):
            for hl in range(2):
                thunks.append(chunk(hl, jp, i))
        for hl in range(2):
            thunks.append(finalize(hl, jp))
    return thunks


def _outproj_thunks(cx, jp):
    """Output projection for t-panel jp over all 16 e-tiles."""
    nc = cx.nc
    thunks = []

    def etile(et):
        def f():
            ps = cx.psO.tile([P, PAN], F32, tag="psO")
            for dc in range(HPC):
                nc.tensor.matmul(
                    ps,
                    lhsT=cx.wo_sb[:, dc, ds(et * P, P)],
                    rhs=cx.y_sb[:, dc, ds(jp * PAN, PAN)],
                    start=(dc == 0),
                    stop=(dc == HPC - 1),
                )
            ob = cx.opool.tile([P, PAN], F32, tag="ob")
            nc.scalar.copy(ob, ps)
            nc.gpsimd.dma_start(
                cx.out[ds(et * P, P), ds(jp * PAN, PAN)], ob
            )
        return f

    for et in range(2 * HPC):
        thunks.append(etile(et))
    return thunks


def make_tables():
    j = np.arange(0, D, 2, dtype=np.float64) / D
    inv_freq = 1.0 / (10000.0 ** j)
    t = np.arange(T, dtype=np.float64)
    fr = np.outer(t, inv_freq)                            # [T, 64]
    c2 = np.cos(2.0 * fr).T                               # [64, T]
    s2 = np.sin(2.0 * fr).T
    cos2 = np.concatenate([c2, c2], axis=0).astype(np.float16)
    sin2 = np.concatenate([s2, -s2], axis=0).astype(np.float16)
    return cos2, sin2


def make_mask():
    import ml_dtypes
    s = np.arange(P)[:, None]
    c = np.arange(P)[None, :]
    return np.where(s <= c, 0.0, NEG).astype(ml_dtypes.bfloat16)


def make_in_maps(x, wq, wk, wv, wo):
    import ml_dtypes
    bf = ml_dtypes.bfloat16
    cos2, sin2 = make_tables()
    mask = make_mask()
    in_maps = []
    for c in range(8):
        b, hh = c // 2, c % 2
        rows = slice(hh * HPC * D, (hh + 1) * HPC * D)
        in_maps.append({
            "xT": np.ascontiguousarray(x[b].T).astype(bf),
            "wqT": np.ascontiguousarray(wq[rows].T).astype(bf),
            "wkT": np.ascontiguousarray(wk[rows].T).astype(bf),
            "wvT": np.ascontiguousarray(wv[rows].T).astype(bf),
            "woT": np.ascontiguousarray(wo[:, rows].T).astype(bf),
            "cos2": cos2,
            "sin2": sin2,
            "mask": mask,
        })
    return in_maps


_PROGRAM_CACHE = {}


def get_program():
    if "nc" not in _PROGRAM_CACHE:
        _PROGRAM_CACHE["nc"] = build_program()
    return _PROGRAM_CACHE["nc"]


def kernel(x, wq, wk, wv, wo, _results_hook=None):
    x = np.asarray(x, dtype=np.float32)
    wq = np.asarray(wq, dtype=np.float32)
    wk = np.asarray(wk, dtype=np.float32)
    wv = np.asarray(wv, dtype=np.float32)
    wo = np.asarray(wo, dtype=np.float32)

    nc = get_program()
    in_maps = make_in_maps(x, wq, wk, wv, wo)
    res = run_bass_kernel_spmd(nc, in_maps, list(range(8)))
    if _results_hook is not None:
        _results_hook(res)
    outs = [r["out"] for r in res.results]
    full = np.empty((B, T, E), dtype=np.float32)
    for b in range(B):
        full[b] = (outs[2 * b] + outs[2 * b + 1]).T
    return full
